# revision 13
# baseline (speedup 1.0000x reference)
"""KMeans (vq_codebook) Trainium2 Bass kernel, data-parallel over samples on 8 cores.

Algorithm (mirrors reference.py):
  - shard data [65536,256] row-wise across 8 cores (8192 rows each); centers replicated
  - per iteration: scores s = x@cT via PE (fp32), argmin distance via DVE max/max_index
    on v = (s - x2/2) - c2/2  (== -d2/2, same rounding as reference's x2 - 2s + c2),
    one-hot segment sums via PE matmul, AllReduce(sums+counts), mean update on-device
  - final: per-center nearest-point argmin over n, combined across cores with
    AllReduce(max) + AllReduce(min) index resolution
Host side only reshapes/shards inputs, gathers rows for init/reinit pools
(index lookup of provided inputs) and concatenates shard outputs.
"""

import numpy as np

import concourse.bass as bass
import concourse.bacc as bacc
import concourse.mybir as mybir
import concourse.tile as tile

N, D, K, ITERS, CORES = 65536, 256, 512, 10, 8
NS = N // CORES          # 8192 samples per core
NT = NS // 128           # 64 tiles of 128 samples
DC = D // 128            # 2 contraction chunks of the feature dim
KC = K // 128            # 4 chunks of centers
NCH = NS // 512          # 16 n-chunks in the final pass
F32 = mybir.dt.float32
U32 = mybir.dt.uint32
U8 = mybir.dt.uint8
Alu = mybir.AluOpType
BIG = 16777216.0         # 2**24, larger than any sample index


def build_nc(finalize=True):
    nc = bacc.Bacc(None, num_devices=CORES, debug=False)

    dn_d = nc.dram_tensor("data_nat", [NS, D], F32, kind="ExternalInput")
    dt_d = nc.dram_tensor("dataT", [D, NS], F32, kind="ExternalInput")
    c0_d = nc.dram_tensor("c0T", [D, K], F32, kind="ExternalInput")
    re_d = nc.dram_tensor("reinitT", [ITERS, D, K], F32, kind="ExternalInput")
    off_d = nc.dram_tensor("core_off", [1, 1], F32, kind="ExternalInput")
    a_out = nc.dram_tensor("a_out", [NS], U32, kind="ExternalOutput")
    ct_out = nc.dram_tensor("ct_out", [D, K], F32, kind="ExternalOutput")
    idx_out = nc.dram_tensor("idx_out", [K], F32, kind="ExternalOutput")

    with tile.TileContext(nc) as tc:
        with (
            tc.tile_pool(name="pers", bufs=1) as pers,
            tc.tile_pool(name="work", bufs=3) as work,
            tc.tile_pool(name="cpool", bufs=2) as cpool,
            tc.tile_pool(name="ser", bufs=1) as ser,
            tc.tile_pool(name="ps", bufs=2, space="PSUM") as ps,
            tc.tile_pool(name="psacc", bufs=1, space="PSUM") as psacc,
            tc.tile_pool(name="dram", bufs=2, space="DRAM") as dpool,
        ):
            # ---------- persistent SBUF state ----------
            dn = pers.tile([128, NT, D], F32, tag="dn")        # data natural [p, t, d]
            dT = pers.tile([128, DC, NS], F32, tag="dT")       # dataT [p(d), c, n]
            x2h = pers.tile([128, NT], F32, tag="x2h")         # x2/2 per sample, [p, t]
            iota = pers.tile([128, K], F32, tag="iota")
            ones_col = pers.tile([128, 1], F32, tag="ones_col")
            ones_row = pers.tile([1, 128], F32, tag="ones_row")
            zeros_row = pers.tile([1, K], F32, tag="zeros_row")
            a_big = pers.tile([128, 8 * NT], U32, tag="a_big")  # max_index lands here
            off_col = pers.tile([128, 1], F32, tag="off_col")  # core offset bcast

            nc.sync.dma_start(dn[:], dn_d.ap().rearrange("(t p) d -> p t d", p=128))
            nc.sync.dma_start(dT[:], dt_d.ap().rearrange("(c p) n -> p c n", p=128))
            nc.gpsimd.iota(iota[:], pattern=[[1, K]], base=0, channel_multiplier=0,
                           allow_small_or_imprecise_dtypes=True)
            nc.vector.memset(ones_col[:], 1.0)
            nc.vector.memset(ones_row[:], 1.0)
            nc.vector.memset(zeros_row[:], 0.0)

            # x2/2 per tile; also a row layout for the final pass
            sq_scr = pers.tile([128, D], F32, tag="sq_scr")
            for t in range(NT):
                x2c = work.tile([128, 1], F32, tag="x2c")
                nc.vector.tensor_mul(sq_scr[:], dn[:, t, :], dn[:, t, :])
                nc.vector.reduce_sum(x2c[:], sq_scr[:], axis=mybir.AxisListType.X)
                nc.vector.tensor_scalar(x2h[:, t : t + 1], x2c[:], 0.5, None, Alu.mult)

            # core offset broadcast to [128,1] via PE outer product
            off_sb = ser.tile([1, 1], F32, tag="off_sb")
            nc.sync.dma_start(off_sb[:], off_d.ap())
            ps_off = ps.tile([128, 1], F32, tag="small")
            nc.tensor.matmul(ps_off[:], ones_row[:], off_sb[:], start=True, stop=True)
            nc.vector.tensor_copy(off_col[:], ps_off[:])

            # initial centers
            cT = cpool.tile([128, DC, K], F32, tag="cT")
            nc.sync.dma_start(cT[:], c0_d.ap().rearrange("(c p) k -> p c k", p=128))

            def c2_half_row(cT_cur, tag):
                """[1,K] row of ||c||^2 / 2 from the transposed centers."""
                ps_c2 = ps.tile([1, K], F32, tag="c2ps")
                for c in range(DC):
                    sq = work.tile([128, K], F32, tag="v")
                    nc.vector.tensor_mul(sq[:], cT_cur[:, c, :], cT_cur[:, c, :])
                    nc.tensor.matmul(ps_c2[:], ones_col[:], sq[:],
                                     start=(c == 0), stop=(c == DC - 1))
                row = ser.tile([1, K], F32, tag=tag)
                nc.vector.tensor_scalar(row[:], ps_c2[:], 0.5, None, Alu.mult)
                return row

            def bcast_row(row, tag):
                """[1,K] -> [128,K] via ones outer product on PE."""
                ps_b = ps.tile([128, K], F32, tag="small")
                nc.tensor.matmul(ps_b[:], ones_row[:], row[:], start=True, stop=True)
                sb = ser.tile([128, K], F32, tag=tag)
                nc.vector.tensor_copy(sb[:], ps_b[:])
                return sb

            # ---------------- main iterations ----------------
            for i in range(ITERS):
                c2hb = bcast_row(c2_half_row(cT, "c2row"), "c2hb")

                sums0 = psacc.tile([128, K], F32, tag="acc0")
                sums1 = psacc.tile([128, K], F32, tag="acc1")
                counts_acc = ser.tile([128, K], F32, tag="cacc")

                for t in range(NT):
                    s_ps = ps.tile([128, K], F32, tag="scores")
                    for c in range(DC):
                        nc.tensor.matmul(
                            s_ps[:], dT[:, c, t * 128 : (t + 1) * 128], cT[:, c, :],
                            start=(c == 0), stop=(c == DC - 1))
                    v = work.tile([128, K], F32, tag="v")
                    nc.vector.scalar_tensor_tensor(
                        out=v[:], in0=s_ps[:], scalar=x2h[:, t : t + 1],
                        in1=c2hb[:], op0=Alu.subtract, op1=Alu.subtract)
                    m8 = work.tile([128, 8], F32, tag="m8")
                    nc.vector.max(m8[:], v[:])
                    nc.vector.max_index(a_big[:, t * 8 : t * 8 + 8], m8[:], v[:])
                    a_f = work.tile([128, 1], F32, tag="a_f")
                    nc.vector.tensor_copy(a_f[:], a_big[:, t * 8 : t * 8 + 1])
                    onehot = work.tile([128, K], F32, tag="onehot")
                    nc.vector.tensor_scalar(
                        onehot[:], iota[:], a_f[:], None, Alu.is_equal)
                    if t == 0:
                        nc.vector.tensor_copy(counts_acc[:], onehot[:])
                    else:
                        nc.vector.tensor_add(counts_acc[:], counts_acc[:], onehot[:])
                    for c in range(DC):
                        nc.tensor.matmul(
                            [sums0, sums1][c][:],
                            dn[:, t, c * 128 : (c + 1) * 128], onehot[:],
                            start=(t == 0), stop=(t == NT - 1))

                cnt_ps = ps.tile([1, K], F32, tag="c2ps")
                nc.tensor.matmul(cnt_ps[:], ones_col[:], counts_acc[:],
                                 start=True, stop=True)

                # pack local [sums ; counts] and AllReduce (PSUM -> SBUF -> DRAM)
                Copy = mybir.ActivationFunctionType.Copy
                sums_sb = ser.tile([128, DC, K], F32, tag="sums_sb")
                cnt_sb = ser.tile([1, K], F32, tag="cnt_sb")
                nc.scalar.activation(sums_sb[:, 0, :], sums0[:], Copy)
                nc.scalar.activation(sums_sb[:, 1, :], sums1[:], Copy)
                nc.scalar.activation(cnt_sb[:], cnt_ps[:], Copy)
                ar_in = dpool.tile([D + 1, K], F32, tag="ar_in")
                ar_out = dpool.tile([D + 1, K], F32, tag="ar_out")
                nc.sync.dma_start(
                    ar_in[0:256, :].rearrange("(c p) k -> p c k", p=128), sums_sb[:])
                nc.sync.dma_start(ar_in[256:257, :], cnt_sb[:])
                nc.gpsimd.collective_compute(
                    "AllReduce", Alu.add,
                    replica_groups=[list(range(CORES))],
                    ins=[ar_in.opt()], outs=[ar_out.opt()])

                g = ser.tile([128, DC, K], F32, tag="g")
                cnt = ser.tile([1, K], F32, tag="cnt")
                nc.sync.dma_start(g[:], ar_out[0:256, :].rearrange("(c p) k -> p c k", p=128))
                nc.sync.dma_start(cnt[:], ar_out[256:257, :])

                cnt1 = ser.tile([1, K], F32, tag="cnt1")
                nc.vector.tensor_scalar_max(cnt1[:], cnt[:], 1.0)
                rr = ser.tile([1, K], F32, tag="rr")
                nc.vector.reciprocal(rr[:], cnt1[:])
                dead = ser.tile([1, K], F32, tag="dead")
                nc.vector.tensor_scalar(dead[:], cnt[:], 0.0, None, Alu.is_equal)
                dead_u8 = ser.tile([1, K], U8, tag="dead_u8")
                nc.vector.tensor_copy(dead_u8[:], dead[:])
                nc.vector.copy_predicated(rr[:], dead_u8[:], zeros_row[:])

                rrb = bcast_row(rr, "rrb")
                deadb = bcast_row(dead, "deadb")

                rein = ser.tile([128, DC, K], F32, tag="rein")
                nc.sync.dma_start(rein[:], re_d[i].rearrange("(c p) k -> p c k", p=128))

                cT_new = cpool.tile([128, DC, K], F32, tag="cT")
                for c in range(DC):
                    nc.vector.tensor_mul(g[:, c, :], g[:, c, :], rrb[:])
                    nc.vector.tensor_mul(rein[:, c, :], rein[:, c, :], deadb[:])
                    nc.vector.tensor_add(cT_new[:, c, :], g[:, c, :], rein[:, c, :])
                cT = cT_new

            # ---------------- outputs of the loop ----------------
            nc.sync.dma_start(ct_out.ap().rearrange("(c p) k -> p c k", p=128), cT[:])
            nc.sync.dma_start(
                a_out.ap().rearrange("(t p) -> p t", p=128),
                a_big[:].rearrange("p (t e) -> p t e", e=8)[:, :, 0])

            # ---------------- final pass: nearest sample per center ----------------
            c2f_row = c2_half_row(cT, "c2row")      # [1,K] of ||c||^2/2
            c2f_col = pers.tile([128, KC], F32, tag="c2fcol")
            for kc in range(KC):
                nc.sync.dma_start(c2f_col[:, kc : kc + 1],
                                  c2f_row[0:1, kc * 128 : (kc + 1) * 128])

            bestv = pers.tile([128, KC], F32, tag="bestv")
            besti = pers.tile([128, KC], F32, tag="besti")

            for nch in range(NCH):
                nlo = nch * 512
                x2row = cpool.tile([1, 512], F32, tag="x2row")
                for j in range(4):
                    nc.sync.dma_start(x2row[0:1, j * 128 : (j + 1) * 128],
                                      x2h[:, nch * 4 + j : nch * 4 + j + 1])
                x2b_ps = ps.tile([128, 512], F32, tag="small")
                nc.tensor.matmul(x2b_ps[:], ones_row[:], x2row[:], start=True, stop=True)
                x2hb = work.tile([128, 512], F32, tag="onehot")
                nc.vector.tensor_copy(x2hb[:], x2b_ps[:])
                for kc in range(KC):
                    f_ps = ps.tile([128, 512], F32, tag="scores")
                    for c in range(DC):
                        nc.tensor.matmul(
                            f_ps[:], cT[:, c, kc * 128 : (kc + 1) * 128],
                            dT[:, c, nlo : nlo + 512],
                            start=(c == 0), stop=(c == DC - 1))
                    w = work.tile([128, 512], F32, tag="v")
                    nc.vector.tensor_sub(w[:], f_ps[:], x2hb[:])
                    nc.vector.tensor_scalar(w[:], w[:], c2f_col[:, kc : kc + 1], None,
                                            Alu.subtract)
                    m8 = work.tile([128, 8], F32, tag="m8")
                    i8 = work.tile([128, 8], U32, tag="i8")
                    nc.vector.max(m8[:], w[:])
                    nc.vector.max_index(i8[:], m8[:], w[:])
                    gi0 = work.tile([128, 1], F32, tag="gi0")
                    nc.vector.tensor_copy(gi0[:], i8[:, 0:1])
                    gi = work.tile([128, 1], F32, tag="gi")
                    nc.vector.tensor_scalar(gi[:], gi0[:], float(nlo), off_col[:],
                                            Alu.add, Alu.add)
                    if nch == 0:
                        nc.vector.tensor_copy(bestv[:, kc : kc + 1], m8[:, 0:1])
                        nc.vector.tensor_copy(besti[:, kc : kc + 1], gi[:])
                    else:
                        msk = work.tile([128, 1], U8, tag="msk")
                        nc.vector.tensor_tensor(msk[:], m8[:, 0:1],
                                                bestv[:, kc : kc + 1], Alu.is_gt)
                        nc.vector.copy_predicated(bestv[:, kc : kc + 1], msk[:], m8[:, 0:1])
                        nc.vector.copy_predicated(besti[:, kc : kc + 1], msk[:], gi[:])

            # cross-core: AllReduce(max) of values, then AllReduce(min) of candidate idx
            arv_in = dpool.tile([128, KC], F32, tag="arv_in")
            arv_out = dpool.tile([128, KC], F32, tag="arv_out")
            nc.sync.dma_start(arv_in[:], bestv[:])
            nc.gpsimd.collective_compute(
                "AllReduce", Alu.max, replica_groups=[list(range(CORES))],
                ins=[arv_in.opt()], outs=[arv_out.opt()])
            gmax = pers.tile([128, KC], F32, tag="gmax")
            nc.sync.dma_start(gmax[:], arv_out[:])

            iseq = pers.tile([128, KC], U8, tag="iseq")
            nc.vector.tensor_tensor(iseq[:], bestv[:], gmax[:], Alu.is_equal)
            cand = pers.tile([128, KC], F32, tag="cand")
            nc.vector.memset(cand[:], BIG)
            nc.vector.copy_predicated(cand[:], iseq[:], besti[:])

            ari_in = dpool.tile([128, KC], F32, tag="ari_in")
            ari_out = dpool.tile([128, KC], F32, tag="ari_out")
            nc.sync.dma_start(ari_in[:], cand[:])
            nc.gpsimd.collective_compute(
                "AllReduce", Alu.min, replica_groups=[list(range(CORES))],
                ins=[ari_in.opt()], outs=[ari_out.opt()])
            gidx = pers.tile([128, KC], F32, tag="gidx")
            nc.sync.dma_start(gidx[:], ari_out[:])
            nc.sync.dma_start(idx_out.ap().rearrange("(c p) -> p c", p=128), gidx[:])

    if finalize:
        nc.finalize()
    return nc


_NC_CACHE = {}


def get_nc(finalize=True):
    key = bool(finalize)
    if key not in _NC_CACHE:
        _NC_CACHE[key] = build_nc(finalize)
    return _NC_CACHE[key]


def kernel(data, init_indices, reinit_indices):
    data = np.ascontiguousarray(np.asarray(data, dtype=np.float32))
    ii = np.asarray(init_indices).astype(np.int64)
    ri = np.asarray(reinit_indices).astype(np.int64)

    c0T = np.ascontiguousarray(data[ii].T)                          # [D, K]
    reinT = np.ascontiguousarray(np.transpose(data[ri], (0, 2, 1)))  # [ITERS, D, K]

    in_maps = []
    for c in range(CORES):
        shard = data[c * NS : (c + 1) * NS]
        in_maps.append({
            "data_nat": np.ascontiguousarray(shard),
            "dataT": np.ascontiguousarray(shard.T),
            "c0T": c0T,
            "reinitT": reinT,
            "core_off": np.array([[c * NS]], dtype=np.float32),
        })

    from concourse.bass_utils import run_bass_kernel_spmd
    nc = get_nc()
    res = run_bass_kernel_spmd(nc, in_maps, core_ids=list(range(CORES)))
    outs = res.results

    a = np.concatenate([outs[c]["a_out"] for c in range(CORES)]).astype(np.int32)
    ct = outs[0]["ct_out"]                       # [D, K]
    avg_center = np.ascontiguousarray(ct.T)      # [K, D]
    index = outs[0]["idx_out"].astype(np.int32)
    center = data[index]
    return (avg_center, a, center, index)
